# revision 60
# baseline (speedup 1.0000x reference)
"""Trainium2 Bass kernel for DilateAttention (3x3 kernel, dilation 2).

Computation (see module docstring in the original nn.Module):
  q,k,v: [B=4, d=384, H=64, W=64] f32.  heads=12, head_dim=32.
  For every pixel l and head n: attend over the 9 dilated neighbors
  (offsets {0,2,4} - 2 in each spatial dim, zero-padded).
  out: [B, H, W, d] f32.

Mapping (per NeuronCore; 8 cores; core = (batch b, row-half)):
  - channel-major layout: 128 partitions = 4 heads x 32 channels (3 groups)
  - 9 shifted elementwise q*k products on DVE (bf16)
  - partition reduction (32->1 per head) via block-diagonal ones matmul on
    TensorE, 9 offsets accumulated into one PSUM tile -> logits [36, N]
  - exp on ScalarE (scale folded in), Z via ones matmul, 1/Z on DVE
    (reciprocal_approx_fast), normalize -> attn [36, N] bf16
  - broadcast attn rows back to 128 partitions via selector matmul,
    multiply with shifted V (DVE), accumulate over 9 offsets
  - PE transpose [128,128] blocks -> pixel-major PSUM f32 -> DMA to DRAM
"""

import os
import sys

for _p in ("/opt/trn_rl_repo", "/root/.axon_site/_ro/trn_rl_repo"):
    if _p not in sys.path and os.path.isdir(_p):
        sys.path.insert(0, _p)

import dataclasses
from contextlib import ExitStack

import numpy as np
import ml_dtypes

import concourse.bass as bass
import concourse.bacc as bacc
import concourse.mybir as mybir
import concourse.tile as tile
from concourse import masks
from concourse.bass_utils import run_bass_kernel_spmd

BF16 = ml_dtypes.bfloat16

# problem constants (hardcoded per spec)
B, D, H, W = 4, 384, 64, 64
NH, HD = 12, 32
KK, DIL, PAD = 3, 2, 2
K2 = KK * KK
SCALE = HD ** -0.5

NCORES = 8
ROWS = H // 2              # 32 output rows per core
HROWS = ROWS + 2 * PAD     # 36 halo rows of padded k/v
WP = W + 2 * PAD           # 68 padded width
NGRP = 3                   # channel groups of 128 partitions
HPG = 4                    # heads per group
FD = ROWS * W              # 2048 pixels per core
NCH = 4                    # psum chunks
CHD = FD // NCH            # 512 chunk free dim
LG = HPG * K2              # 36 logit rows per group

_CACHE = {}


def _build_sel_constants():
    """Selector/ones matrices used as TensorE stationary operands."""
    # QK reduce: for offset ko, lhsT[p=hl*32+c, m=hl*9+ko] = 1
    selqk = np.zeros((128, K2, LG), np.float32)
    for hl in range(HPG):
        for c in range(HD):
            for ko in range(K2):
                selqk[hl * HD + c, ko, hl * K2 + ko] = 1.0
    # Z: lhsT[p=hl*9+ko, m=hl] = 1
    selz = np.zeros((LG, HPG), np.float32)
    for hl in range(HPG):
        for ko in range(K2):
            selz[hl * K2 + ko, hl] = 1.0
    # Z broadcast back: lhsT[p=hl, m=hl*9+ko] = 1  (f32; rhs is f32 1/Z)
    selzt = selz.T.copy()
    # attn broadcast: for offset ko, lhsT[p=hl*9+ko, m=hl*32+c] = 1
    selbc = np.zeros((LG, K2, 128), np.float32)
    for hl in range(HPG):
        for ko in range(K2):
            for c in range(HD):
                selbc[hl * K2 + ko, ko, hl * HD + c] = 1.0
    return (
        selqk.reshape(128, K2 * LG).astype(BF16),
        selz.astype(BF16),
        selzt.astype(np.float32),
        selbc.reshape(LG, K2 * 128).astype(BF16),
    )


def _build_nc():
    nc = bacc.Bacc("TRN2", target_bir_lowering=False, debug=False,
                   num_devices=NCORES)
    f32 = mybir.dt.float32
    bf16 = mybir.dt.bfloat16

    q_p = nc.declare_dram_parameter("q", [D, FD], bf16, isOutput=False)
    k_p = nc.declare_dram_parameter("k", [D, HROWS * WP], bf16, isOutput=False)
    v_p = nc.declare_dram_parameter("v", [D, HROWS * WP], bf16, isOutput=False)
    selqk_p = nc.declare_dram_parameter("selqk", [128, K2 * LG], bf16, isOutput=False)
    selz_p = nc.declare_dram_parameter("selz", [LG, HPG], bf16, isOutput=False)
    selbc_p = nc.declare_dram_parameter("selbc", [LG, K2 * 128], bf16, isOutput=False)
    out_p = nc.declare_dram_parameter("out", [FD, D], f32, isOutput=True)

    with tile.TileContext(nc) as tc, ExitStack() as ctx:
        consts = ctx.enter_context(tc.tile_pool(name="consts", bufs=1))
        inp = ctx.enter_context(tc.tile_pool(name="inp", bufs=2))
        prods = ctx.enter_context(tc.tile_pool(name="prods", bufs=2))
        smax = ctx.enter_context(tc.tile_pool(name="smax", bufs=3))
        accp = ctx.enter_context(tc.tile_pool(name="accp", bufs=2))
        ps_sm = ctx.enter_context(tc.tile_pool(name="ps_sm", bufs=2, space="PSUM"))
        ps_bc = ctx.enter_context(tc.tile_pool(name="ps_bc", bufs=4, space="PSUM"))
        ps_tx = ctx.enter_context(tc.tile_pool(name="ps_tx", bufs=2, space="PSUM"))
        ps_lg = ps_zz = ps_sm

        # constants
        selqk_t = consts.tile([128, K2 * LG], bf16)
        nc.sync.dma_start(selqk_t[:], selqk_p[:])
        selz_t = consts.tile([LG, HPG], bf16)
        nc.sync.dma_start(selz_t[:], selz_p[:])
        selbc_t = consts.tile([LG, K2 * 128], bf16)
        nc.sync.dma_start(selbc_t[:], selbc_p[:])
        ident = consts.tile([128, 128], bf16)
        masks.make_identity(nc, ident[:])
        identf = consts.tile([4, 4], bf16)
        masks.make_identity(nc, identf[:])

        def shifted(t3, ko, ch=None):
            """AP into padded [128, HROWS, WP] tile for offset ko, chunk ch."""
            di, dj = divmod(ko, KK)
            rows = slice(ch * (ROWS // NCH), (ch + 1) * (ROWS // NCH)) if ch is not None \
                else slice(0, ROWS)
            r0 = rows.start
            nr = rows.stop - rows.start
            return t3[:, DIL * di + r0: DIL * di + r0 + nr, DIL * dj: DIL * dj + W]

        PCD = 512            # pixels per AV chunk
        NPC = FD // PCD      # 4 chunks
        RPC = PCD // W       # 8 rows per chunk
        ACT_KO = (0, 1, 3, 4, 6, 8)   # bc copied to SBUF by ScalarE

        def load_group(g):
            gp = slice(g * 128, (g + 1) * 128)
            q_t = inp.tile([128, FD], bf16, tag="q", name="q_t")
            nc.sync.dma_start(q_t[:], q_p[gp, :])
            k_t = inp.tile([128, HROWS * WP], bf16, tag="k", name="k_t")
            nc.sync.dma_start(k_t[:], k_p[gp, :])
            v_t = inp.tile([128, HROWS * WP], bf16, tag="v", name="v_t")
            nc.sync.dma_start(v_t[:], v_p[gp, :])
            k3 = k_t[:].rearrange("p (r w) -> p r w", r=HROWS)
            v3 = v_t[:].rearrange("p (r w) -> p r w", r=HROWS)
            return q_t, k3, v3

        def dj_triple(t3, di, r0, nr):
            """AP [128, (3 dj: step 2), (nr rows), (W cols)] into padded tile."""
            sl = t3[:, DIL * di + r0: DIL * di + r0 + nr, 0:W]
            return dataclasses.replace(
                sl, ap=[sl.ap[0], [DIL, KK]] + list(sl.ap[1:]))

        def rep3(flat, nr):
            """[128, nr*W] contiguous -> [128, (3: step 0), nr, W]."""
            sl = flat.rearrange("p (r w) -> p r w", r=nr)
            return dataclasses.replace(
                sl, ap=[sl.ap[0], [0, KK]] + list(sl.ap[1:]))

        def emit_prods(q_t, k3):
            # 9 shifted q*k products as 3 dj-paired DVE ops (bf16 2x mode)
            ptiles = []
            for di in range(KK):
                pt = prods.tile([128, KK * FD], bf16, tag=f"pd{di}",
                                name=f"pd{di}")
                nc.vector.tensor_mul(
                    pt[:].rearrange("p (k r w) -> p k r w", k=KK, r=ROWS),
                    rep3(q_t[:], ROWS),
                    dj_triple(k3, di, 0, ROWS),
                )
                ptiles.extend(pt[:, dj * FD:(dj + 1) * FD] for dj in range(KK))
            return ptiles

        def emit_softmax(ptiles):
            exp_t = smax.tile([LG, FD], bf16, tag="exp", name="exp_t")
            zr_t = smax.tile([HPG, FD], bf16, tag="zr", name="zr_t")
            for ch in range(NCH):
                cs = slice(ch * CHD, (ch + 1) * CHD)
                lg = ps_lg.tile([LG, CHD], f32, tag="sm", name="lg")
                for ko in range(K2):
                    nc.tensor.matmul(
                        lg[:],
                        selqk_t[:, ko * LG:(ko + 1) * LG],
                        ptiles[ko][:, cs],
                        start=(ko == 0),
                        stop=(ko == K2 - 1),
                    )
                # exp(scale * logits) -> bf16
                nc.scalar.activation(
                    exp_t[:, cs], lg[:], mybir.ActivationFunctionType.Exp,
                    scale=float(SCALE),
                )
            for ch in range(NCH):
                cs = slice(ch * CHD, (ch + 1) * CHD)
                zp = ps_zz.tile([LG, CHD], f32, tag="sm", name="zp")
                nc.tensor.matmul(zp[:HPG, :], selz_t[:], exp_t[:, cs],
                                 start=True, stop=True)
                # Z downcast to bf16 (reciprocal happens post-transpose)
                nc.vector.tensor_copy(zr_t[:, cs], zp[:HPG, :])
            return exp_t, zr_t

        def emit_zrt(zrb):
            # transpose Z to pixel-major, then 1/Z on the small tile:
            # zrt[p, t*4+h] = 1 / Z[h, t*128+p]
            zrt_ps = ps_zz.tile([128, (FD // 128) * HPG], f32, tag="sm",
                                name="zrt_ps")
            for t in range(FD // 128):
                nc.tensor.matmul(zrt_ps[:, t * HPG:(t + 1) * HPG],
                                 zrb[:, t * 128:(t + 1) * 128], identf[:],
                                 start=True, stop=True)
            zrt = smax.tile([128, (FD // 128) * HPG], f32, tag="zrt",
                            name="zrt")
            nc.vector.reciprocal_approx_fast(zrt[:], zrt_ps[:])
            return zrt

        def emit_av_chunk(g, pc, exp_t, v3, zrt_holder):
            gp = slice(g * 128, (g + 1) * 128)
            pavs = []
            r0 = pc * RPC
            for di in range(KK):
                bcs3 = smax.tile([128, KK * PCD], bf16, tag="bcs3",
                                 name="bcs3")
                for dj in range(KK):
                    ko = di * KK + dj
                    bc = ps_bc.tile([128, PCD], f32, tag="bc", name="bc")
                    for h in range(PCD // CHD):
                        nc.tensor.matmul(
                            bc[:, h * CHD:(h + 1) * CHD],
                            selbc_t[:, ko * 128:(ko + 1) * 128],
                            exp_t[:, pc * PCD + h * CHD: pc * PCD + (h + 1) * CHD],
                            start=True, stop=True,
                        )
                    nc.scalar.copy(bcs3[:, dj * PCD:(dj + 1) * PCD], bc[:])
                pav = accp.tile([128, KK * PCD], bf16, tag=f"pav3_{di}",
                                name=f"pav3_{di}")
                nc.vector.tensor_mul(
                    pav[:].rearrange("p (k r w) -> p k r w", k=KK, r=RPC),
                    bcs3[:].rearrange("p (k r w) -> p k r w", k=KK, r=RPC),
                    dj_triple(v3, di, r0, RPC),
                )
                pavs.extend(pav[:, dj * PCD:(dj + 1) * PCD] for dj in range(KK))
            # sum over k on TensorE (identity stationary, PSUM accumulate)
            acc_ps = ps_tx.tile([128, PCD], f32, tag="tx", name="acc_ps")
            for h in range(PCD // CHD):
                hs = slice(h * CHD, (h + 1) * CHD)
                for ko in range(K2):
                    nc.tensor.matmul(acc_ps[:, hs], ident[:],
                                     pavs[ko][:, hs],
                                     start=(ko == 0), stop=(ko == K2 - 1))
            accs = smax.tile([128, PCD], bf16, tag="accs", name="accs")
            nc.scalar.copy(accs[:], acc_ps[:])
            if zrt_holder and zrt_holder[0] is None:
                # zrT transposes emitted here: past the first bc/ksum stream
                # so they never head-of-line block ready broadcast matmuls
                zrt_holder[0] = emit_zrt(zrt_holder[1])
            zrt = zrt_holder[0]
            # transpose to pixel-major
            tp = ps_tx.tile([128, PCD], f32, tag="tx", name="tp")
            for j in range(PCD // 128):
                js = slice(j * 128, (j + 1) * 128)
                nc.tensor.matmul(tp[:, js], accs[:, js], ident[:],
                                 start=True, stop=True)
            # fused normalization: st[p, j, h, c] = tp * zr(pixel-major)
            st = smax.tile([128, PCD], f32, tag="st", name="st")
            nj = PCD // 128
            zfac = zrt[:, pc * nj * HPG:(pc + 1) * nj * HPG].rearrange(
                "p (j h) -> p j h", j=nj).broadcast_to((128, nj, HPG, HD))
            nc.vector.tensor_mul(
                st[:].rearrange("p (j h c) -> p j h c", j=nj, h=HPG),
                tp[:].rearrange("p (j h c) -> p j h c", j=nj, h=HPG),
                zfac,
            )
            dst = out_p[pc * PCD:(pc + 1) * PCD, gp].rearrange(
                "(j p) d -> p j d", p=128)
            nc.sync.dma_start(dst, st[:].rearrange("p (j d) -> p j d",
                                                   j=PCD // 128))

        # software pipeline over the 3 channel groups: the next group's
        # q*k products run on DVE/GPSIMD while TensorE streams the current
        # group's broadcast/k-sum matmuls.
        tiles = load_group(0)
        ptiles = emit_prods(tiles[0], tiles[1])
        for g in range(NGRP):
            v3_cur = tiles[2]
            exp_t, zrb = emit_softmax(ptiles)
            if g + 1 < NGRP:
                tiles = load_group(g + 1)
                ptiles = emit_prods(tiles[0], tiles[1])
            zrt_holder = [None, zrb]
            for pc in range(NPC):
                emit_av_chunk(g, pc, exp_t, v3_cur, zrt_holder)

    nc.compile()
    return nc


def _get_nc():
    if "nc" not in _CACHE:
        _CACHE["nc"] = _build_nc()
    return _CACHE["nc"]


def kernel(q, k, v):
    q = np.asarray(q, dtype=np.float32)
    k = np.asarray(k, dtype=np.float32)
    v = np.asarray(v, dtype=np.float32)

    qb = q.astype(BF16)
    kp = np.pad(k, ((0, 0), (0, 0), (PAD, PAD), (PAD, PAD))).astype(BF16)
    vp = np.pad(v, ((0, 0), (0, 0), (PAD, PAD), (PAD, PAD))).astype(BF16)

    selqk, selz, selzt, selbc = _CACHE.setdefault("sel", _build_sel_constants())  # noqa

    in_maps = []
    for c in range(NCORES):
        b, half = divmod(c, 2)
        r0 = half * ROWS
        in_maps.append({
            "q": np.ascontiguousarray(qb[b, :, r0:r0 + ROWS, :]).reshape(D, FD),
            "k": np.ascontiguousarray(kp[b, :, r0:r0 + HROWS, :]).reshape(D, HROWS * WP),
            "v": np.ascontiguousarray(vp[b, :, r0:r0 + HROWS, :]).reshape(D, HROWS * WP),
            "selqk": selqk, "selz": selz, "selbc": selbc,
        })

    nc = _get_nc()
    res = run_bass_kernel_spmd(nc, in_maps, core_ids=list(range(NCORES)))

    out = np.empty((B, H, W, D), np.float32)
    for c in range(NCORES):
        b, half = divmod(c, 2)
        r0 = half * ROWS
        out[b, r0:r0 + ROWS] = res.results[c]["out"].reshape(ROWS, W, D)
    return out
